# revision 15
# baseline (speedup 1.0000x reference)
"""Cosine-similarity batch attention on 8 TRN2 NeuronCores.

reference:  xn = x / ||x||_row;  out = softmax(xn @ xn.T, axis=-1) @ x
x: [8192, 512] fp32.

Sharding: rows of x (the query dim) are split across the 8 cores; every core
holds the full x for the key/value side.  Per core (SPMD program):

  prep:  load x tiles [128, 512] fp32 (SWDGE/gpsimd queue), batched row
         norms via ACT(Square, accum_out) + Sqrt + Ln, DVE reciprocal,
         scale to fp16 xn stored as V in natural layout, one XBAR
         dma-transpose per tile (Sync queue) into xnT [128, 4c, 8192k].
         Same for this core's 1024 query rows -> qnT [128, 4c, 1024q].
  main:  flash-attention style over 64 k-blocks per 512-wide q-block:
         ST[k,q]  = sum_c xnT[c,kblk].T @ qnT[c,qblk]     (PSUM fp32)
         E'       = exp(ST + ln||k||)  = exp(ST)*||k||    (ACT bias, fp16)
         O[q,C]  += E'[:,qsub].T @ xn[kblk]   ( = exp(ST).T @ x )
         rs[1,q] += rnorm.T @ E'              ( = sum_k exp(ST) )
         The ln||k|| bias folds the un-normalization of V into the scores
         so V is just xn (no separate fp16 cast of x), and weighting the
         denominator colsum by 1/||k|| recovers the plain exp sum.
         Scores are cosines in [-1,1]: no max subtraction needed.
  epi:   transpose rs to [q,1] via K=1 matmuls, out = O * (1/rs), fp32.

All matmul operands fp16 (PE full rate), all accumulation fp32.
"""

import numpy as np

B, C = 8192, 512
M = 8                 # cores
QB = B // M           # 1024 query rows per core
P = 128               # SBUF partitions
NK = B // P           # 64 k-blocks
QBLK = 512            # q-block width (one PSUM bank of fp32)
NQB = QB // QBLK      # 2 q-blocks per core
NSUB = QBLK // P      # 4 q sub-slices per q-block (matmul M<=128)
CCH = C // P          # 4 contraction chunks of 128
NQT = QB // P         # 8 q tiles per core
NGRP = 8              # row-tiles per batched-norm group

_cached_nc = None


def _build():
    import concourse.bacc as bacc
    import concourse.tile as tile
    from concourse import mybir

    f32 = mybir.dt.float32
    f16 = mybir.dt.float16
    Act = mybir.ActivationFunctionType

    nc = bacc.Bacc("TRN2", target_bir_lowering=False, debug=False, num_devices=M)
    # Each core receives x rotated so that its own 1024 query rows are rows
    # 0..1023: attention is permutation-invariant over keys, so the rotated
    # key/value order changes nothing, and the query side becomes a plain
    # view of xnT (no separate query prep).
    x = nc.dram_tensor("x", [B, C], f32, kind="ExternalInput").ap()
    out = nc.dram_tensor("out", [QB, C], f32, kind="ExternalOutput").ap()

    with tile.TileContext(nc) as tc:
        with (
            tc.tile_pool(name="resident", bufs=1) as resident,
            # io holds two norm-batches of NGRP row tiles alive plus slack
            tc.tile_pool(name="io", bufs=NGRP + 2) as io,
            tc.tile_pool(name="work", bufs=4) as work,
            tc.tile_pool(name="nrm", bufs=2) as nrm_pool,
            tc.tile_pool(name="epi", bufs=3) as epi,
            tc.tile_pool(name="st_psum", bufs=2, space="PSUM") as st_psum,
            tc.tile_pool(name="o_psum", bufs=1, space="PSUM") as o_psum,
            tc.tile_pool(name="rs_psum", bufs=1, space="PSUM") as rs_psum,
            tc.tile_pool(name="rst_psum", bufs=1, space="PSUM") as rst_psum,
        ):
            # resident fp16 operand buffers
            xnT = resident.tile([P, CCH, B], f16, name="xnT")
            qnT = xnT
            v = resident.tile([P, NK, C], f16, name="v")           # xn natural
            lognrm = resident.tile([P, NK], f32, name="lognrm")    # ln||k||
            rnorm16 = resident.tile([P, NK], f16, name="rnorm16")  # 1/||k||
            one32 = resident.tile([1, 1], f32, name="one32")
            nc.vector.memset(one32, 1.0)

            def emit_loads(src, g0, n):
                """Load n row tiles, two per DMA, returning per-tile views."""
                xts = []
                for i in range(0, n, 2):
                    r0 = (g0 + i) * P
                    xt2 = io.tile([P, 2, C], f32, tag="xload", name="xt2")
                    nc.scalar.dma_start(
                        out=xt2,
                        in_=src[r0 : r0 + 2 * P, :].rearrange(
                            "(j p) c -> p j c", p=P
                        ),
                    )
                    xts.append(xt2[:, 0, :])
                    xts.append(xt2[:, 1, :])
                return xts

            def prep_norms(xts, g0, n, is_k_side):
                """Row norms of n loaded tiles via DVE bn_stats; rnorm fp32."""
                mv = nrm_pool.tile([P, 2, n], f32, tag="mv")
                for i in range(n):
                    stats = work.tile([P, 6], f32, tag="stats", bufs=2)
                    nc.vector.bn_stats(out=stats, in_=xts[i])
                    nc.vector.bn_aggr(out=mv[:, :, i], in_=stats)
                ssqn = nrm_pool.tile([P, n], f32, tag="ssqn")
                nc.vector.tensor_mul(ssqn, mv[:, 0, :], mv[:, 0, :])
                nc.vector.tensor_add(ssqn, ssqn, mv[:, 1, :])
                nrm = nrm_pool.tile([P, n], f32, tag="nrm")
                # ssqn = sum(x^2)/C, so sqrt(C * ssqn) = ||x||
                nc.scalar.activation(
                    out=nrm, in_=ssqn, func=Act.Sqrt, scale=float(C)
                )
                if is_k_side:
                    nc.scalar.activation(
                        out=lognrm[:, g0 : g0 + n], in_=nrm, func=Act.Ln
                    )
                rnorm = nrm_pool.tile([P, n], f32, tag="rnorm")
                nc.vector.reciprocal(out=rnorm, in_=nrm)
                if is_k_side:
                    nc.vector.tensor_copy(
                        out=rnorm16[:, g0 : g0 + n], in_=rnorm
                    )
                return rnorm

            def prep_scale_transpose(xts, rnorm, g0, n, dest, is_k_side):
                for i in range(n):
                    kb = g0 + i
                    if is_k_side:
                        xnf = v[:, kb, :]
                    else:
                        xnf = work.tile([P, C], f16, tag="xnf", bufs=2)
                    nc.vector.tensor_scalar_mul(
                        out=xnf, in0=xts[i], scalar1=rnorm[:, i : i + 1]
                    )
                    nc.sync.dma_start_transpose(
                        out=dest[:, :, kb * P : (kb + 1) * P], in_=xnf
                    )

            def prep_rest(xts, g0, n, dest, is_k_side):
                rnorm = prep_norms(xts, g0, n, is_k_side)
                prep_scale_transpose(xts, rnorm, g0, n, dest, is_k_side)

            # ---- main-loop building blocks ----
            qb_psum = {}

            def main_iter(qb, kb):
                o_ps, rs_ps = qb_psum[qb]
                st = st_psum.tile([P, QBLK], f32, tag="st")
                for c in range(CCH):
                    nc.tensor.matmul(
                        st,
                        lhsT=xnT[:, c, kb * P : (kb + 1) * P],
                        rhs=qnT[:, c, qb * QBLK : (qb + 1) * QBLK],
                        start=(c == 0),
                        stop=(c == CCH - 1),
                    )
                est = work.tile([P, QBLK], f16, tag="est", bufs=4)
                nc.scalar.activation(
                    out=est, in_=st, func=Act.Exp,
                    bias=lognrm[:, kb : kb + 1],
                )
                for s in range(NSUB):
                    nc.tensor.matmul(
                        o_ps[:, s, :],
                        lhsT=est[:, s * P : (s + 1) * P],
                        rhs=v[:, kb, :],
                        start=(kb == 0),
                        stop=(kb == NK - 1),
                    )
                nc.tensor.matmul(
                    rs_ps,
                    lhsT=rnorm16[:, kb : kb + 1],
                    rhs=est,
                    start=(kb == 0),
                    stop=(kb == NK - 1),
                )

            def epilogue(qb):
                o_ps, rs_ps = qb_psum[qb]
                rs_sb = epi.tile([1, QBLK], f32, tag="rs_sb")
                nc.vector.tensor_copy(out=rs_sb, in_=rs_ps)
                rst_ps = rst_psum.tile([P, NSUB], f32, tag="rst")
                # K=1 fp32 matmuls transpose rs rows into partitions; they
                # share one PSUM bank so zero it once and accumulate.
                nc.vector.memset(rst_ps, 0.0)
                for s in range(NSUB):
                    nc.tensor.matmul(
                        rst_ps[:, s : s + 1],
                        lhsT=rs_sb[0:1, s * P : (s + 1) * P],
                        rhs=one32,
                        start=False,
                        stop=True,
                        skip_group_check=True,
                    )
                recip = epi.tile([P, NSUB], f32, tag="recip")
                nc.vector.reciprocal(out=recip, in_=rst_ps)
                for s in range(NSUB):
                    oo = epi.tile([P, C], f32, tag="oout", bufs=2)
                    nc.vector.tensor_scalar_mul(
                        out=oo, in0=o_ps[:, s, :], scalar1=recip[:, s : s + 1]
                    )
                    r0 = qb * QBLK + s * P
                    nc.gpsimd.dma_start(out=out[r0 : r0 + P, :], in_=oo)

            # ---- software-pipelined emission ----
            # Groups: q tiles first, then the 8 x-tile groups.  Loads for
            # group g+1 are emitted before group g's norm/transpose chain so
            # the in-order Sync queue never stalls the next group's loads,
            # and qb0's main iterations are interleaved group-wise so the
            # in-order ACT queue alternates prep work and exps.
            qb_psum[0] = (
                o_psum.tile([P, NSUB, C], f32, tag="o", name="o_ps0"),
                rs_psum.tile([1, QBLK], f32, tag="rs", name="rs_ps0"),
            )
            NXG = NK // NGRP
            # Two-group pipeline distance between prep and main so the
            # transposes for k-block group g are in SBUF well before the
            # PE consumes them.
            loads = {}
            loads[0] = emit_loads(x, 0, NGRP)
            loads[1] = emit_loads(x, NGRP, NGRP)
            prep_rest(loads.pop(0), 0, NGRP, xnT, is_k_side=True)
            loads[2] = emit_loads(x, 2 * NGRP, NGRP)
            prep_rest(loads.pop(1), NGRP, NGRP, xnT, is_k_side=True)
            for g in range(NXG):
                pg = g + 2
                if pg + 1 < NXG:
                    loads[pg + 1] = emit_loads(x, (pg + 1) * NGRP, NGRP)
                if pg < NXG:
                    prep_rest(
                        loads.pop(pg), pg * NGRP, NGRP, xnT, is_k_side=True
                    )
                for kb in range(g * NGRP, (g + 1) * NGRP):
                    main_iter(0, kb)
            epilogue(0)

            qb_psum[1] = (
                o_psum.tile([P, NSUB, C], f32, tag="o", name="o_ps1"),
                rs_psum.tile([1, QBLK], f32, tag="rs", name="rs_ps1"),
            )
            for kb in range(NK):
                main_iter(1, kb)
            epilogue(1)

    nc.compile()
    return nc


def kernel(**inputs):
    global _cached_nc
    from concourse import bass_utils

    x = np.ascontiguousarray(np.asarray(inputs["x"], dtype=np.float32))
    if _cached_nc is None:
        _cached_nc = _build()
    in_maps = [
        {"x": x if i == 0 else np.concatenate([x[i * QB :], x[: i * QB]])}
        for i in range(M)
    ]
    res = bass_utils.run_bass_kernel_spmd(_cached_nc, in_maps, core_ids=list(range(M)))
    return np.concatenate([res.results[i]["out"] for i in range(M)], axis=0)


# revision 16
# speedup vs baseline: 1.1441x; 1.1441x over previous
"""Cosine-similarity batch attention on 8 TRN2 NeuronCores.

reference:  xn = x / ||x||_row;  out = softmax(xn @ xn.T, axis=-1) @ x
x: [8192, 512] fp32.

Sharding: rows of x (the query dim) are split across the 8 cores; every core
holds the full x for the key/value side.  Per core (SPMD program):

  prep:  load x tiles [128, 512] fp32 (SWDGE/gpsimd queue), batched row
         norms via ACT(Square, accum_out) + Sqrt + Ln, DVE reciprocal,
         scale to fp16 xn stored as V in natural layout, one XBAR
         dma-transpose per tile (Sync queue) into xnT [128, 4c, 8192k].
         Same for this core's 1024 query rows -> qnT [128, 4c, 1024q].
  main:  flash-attention style over 64 k-blocks per 512-wide q-block:
         ST[k,q]  = sum_c xnT[c,kblk].T @ qnT[c,qblk]     (PSUM fp32)
         E'       = exp(ST + ln||k||)  = exp(ST)*||k||    (ACT bias, fp16)
         O[q,C]  += E'[:,qsub].T @ xn[kblk]   ( = exp(ST).T @ x )
         rs[1,q] += rnorm.T @ E'              ( = sum_k exp(ST) )
         The ln||k|| bias folds the un-normalization of V into the scores
         so V is just xn (no separate fp16 cast of x), and weighting the
         denominator colsum by 1/||k|| recovers the plain exp sum.
         Scores are cosines in [-1,1]: no max subtraction needed.
  epi:   transpose rs to [q,1] via K=1 matmuls, out = O * (1/rs), fp32.

All matmul operands fp16 (PE full rate), all accumulation fp32.
"""

import numpy as np

B, C = 8192, 512
M = 8                 # cores
QB = B // M           # 1024 query rows per core
P = 128               # SBUF partitions
NK = B // P           # 64 k-blocks
QBLK = 512            # q-block width (one PSUM bank of fp32)
NQB = QB // QBLK      # 2 q-blocks per core
NSUB = QBLK // P      # 4 q sub-slices per q-block (matmul M<=128)
CCH = C // P          # 4 contraction chunks of 128
NQT = QB // P         # 8 q tiles per core
NGRP = 8              # row-tiles per batched-norm group

_cached_nc = None


def _build():
    import concourse.bacc as bacc
    import concourse.tile as tile
    from concourse import mybir

    f32 = mybir.dt.float32
    f16 = mybir.dt.float16
    Act = mybir.ActivationFunctionType

    nc = bacc.Bacc("TRN2", target_bir_lowering=False, debug=False, num_devices=M)
    # Each core receives x rotated so that its own 1024 query rows are rows
    # 0..1023: attention is permutation-invariant over keys, so the rotated
    # key/value order changes nothing, and the query side becomes a plain
    # view of xnT (no separate query prep).
    x = nc.dram_tensor("x", [B, C], f32, kind="ExternalInput").ap()
    out = nc.dram_tensor("out", [QB, C], f32, kind="ExternalOutput").ap()

    with tile.TileContext(nc) as tc:
        with (
            tc.tile_pool(name="resident", bufs=1) as resident,
            # io holds two norm-batches of NGRP row tiles alive plus slack
            tc.tile_pool(name="io", bufs=12) as io,
            tc.tile_pool(name="work", bufs=4) as work,
            tc.tile_pool(name="nrm", bufs=2) as nrm_pool,
            tc.tile_pool(name="epi", bufs=3) as epi,
            tc.tile_pool(name="st_psum", bufs=2, space="PSUM") as st_psum,
            tc.tile_pool(name="o_psum", bufs=1, space="PSUM") as o_psum,
            tc.tile_pool(name="rs_psum", bufs=1, space="PSUM") as rs_psum,
            tc.tile_pool(name="rst_psum", bufs=1, space="PSUM") as rst_psum,
        ):
            # resident fp16 operand buffers
            xnT = resident.tile([P, CCH, B], f16, name="xnT")
            qnT = xnT
            v = resident.tile([P, NK, C], f16, name="v")           # xn natural
            lognrm = resident.tile([P, NK], f32, name="lognrm")    # ln||k||
            rnorm16 = resident.tile([P, NK], f16, name="rnorm16")  # 1/||k||
            one32 = resident.tile([1, 1], f32, name="one32")
            nc.vector.memset(one32, 1.0)

            def emit_loads(src, g0, n):
                """Load n row tiles, two per DMA, returning per-tile views."""
                xts = []
                for i in range(0, n, 2):
                    r0 = (g0 + i) * P
                    xt2 = io.tile([P, 2, C], f32, tag="xload", name="xt2")
                    nc.sync.dma_start(
                        out=xt2,
                        in_=src[r0 : r0 + 2 * P, :].rearrange(
                            "(j p) c -> p j c", p=P
                        ),
                    )
                    xts.append(xt2[:, 0, :])
                    xts.append(xt2[:, 1, :])
                return xts

            def prep_norms(xts, g0, n, is_k_side):
                """Row norms of n loaded tiles via DVE bn_stats; rnorm fp32."""
                mv = nrm_pool.tile([P, 2, n], f32, tag="mv")
                for i in range(n):
                    stats = work.tile([P, 6], f32, tag="stats", bufs=2)
                    nc.vector.bn_stats(out=stats, in_=xts[i])
                    nc.vector.bn_aggr(out=mv[:, :, i], in_=stats)
                ssqn = nrm_pool.tile([P, n], f32, tag="ssqn")
                nc.vector.tensor_mul(ssqn, mv[:, 0, :], mv[:, 0, :])
                nc.vector.tensor_add(ssqn, ssqn, mv[:, 1, :])
                nrm = nrm_pool.tile([P, n], f32, tag="nrm")
                # ssqn = sum(x^2)/C, so sqrt(C * ssqn) = ||x||
                nc.scalar.activation(
                    out=nrm, in_=ssqn, func=Act.Sqrt, scale=float(C)
                )
                if is_k_side:
                    nc.scalar.activation(
                        out=lognrm[:, g0 : g0 + n], in_=nrm, func=Act.Ln
                    )
                rnorm = nrm_pool.tile([P, n], f32, tag="rnorm")
                nc.vector.reciprocal(out=rnorm, in_=nrm)
                if is_k_side:
                    nc.vector.tensor_copy(
                        out=rnorm16[:, g0 : g0 + n], in_=rnorm
                    )
                return rnorm

            def prep_scale_transpose(xts, rnorm, g0, n, dest, is_k_side):
                for i in range(n):
                    kb = g0 + i
                    if is_k_side:
                        xnf = v[:, kb, :]
                    else:
                        xnf = work.tile([P, C], f16, tag="xnf", bufs=2)
                    nc.vector.tensor_scalar_mul(
                        out=xnf, in0=xts[i], scalar1=rnorm[:, i : i + 1]
                    )
                    nc.sync.dma_start_transpose(
                        out=dest[:, :, kb * P : (kb + 1) * P], in_=xnf
                    )

            def prep_rest(xts, g0, n, dest, is_k_side):
                rnorm = prep_norms(xts, g0, n, is_k_side)
                prep_scale_transpose(xts, rnorm, g0, n, dest, is_k_side)

            # ---- main-loop building blocks ----
            qb_psum = {}

            def main_iter(qb, kb):
                o_ps, rs_ps = qb_psum[qb]
                st = st_psum.tile([P, QBLK], f32, tag="st")
                for c in range(CCH):
                    nc.tensor.matmul(
                        st,
                        lhsT=xnT[:, c, kb * P : (kb + 1) * P],
                        rhs=qnT[:, c, qb * QBLK : (qb + 1) * QBLK],
                        start=(c == 0),
                        stop=(c == CCH - 1),
                    )
                est = work.tile([P, QBLK], f16, tag="est", bufs=4)
                nc.scalar.activation(
                    out=est, in_=st, func=Act.Exp,
                    bias=lognrm[:, kb : kb + 1],
                )
                for s in range(NSUB):
                    nc.tensor.matmul(
                        o_ps[:, s, :],
                        lhsT=est[:, s * P : (s + 1) * P],
                        rhs=v[:, kb, :],
                        start=(kb == 0),
                        stop=(kb == NK - 1),
                    )
                nc.tensor.matmul(
                    rs_ps,
                    lhsT=rnorm16[:, kb : kb + 1],
                    rhs=est,
                    start=(kb == 0),
                    stop=(kb == NK - 1),
                )

            def epilogue(qb):
                o_ps, rs_ps = qb_psum[qb]
                rs_sb = epi.tile([1, QBLK], f32, tag="rs_sb")
                nc.vector.tensor_copy(out=rs_sb, in_=rs_ps)
                rst_ps = rst_psum.tile([P, NSUB], f32, tag="rst")
                # K=1 fp32 matmuls transpose rs rows into partitions; they
                # share one PSUM bank so zero it once and accumulate.
                nc.vector.memset(rst_ps, 0.0)
                for s in range(NSUB):
                    nc.tensor.matmul(
                        rst_ps[:, s : s + 1],
                        lhsT=rs_sb[0:1, s * P : (s + 1) * P],
                        rhs=one32,
                        start=False,
                        stop=True,
                        skip_group_check=True,
                    )
                recip = epi.tile([P, NSUB], f32, tag="recip")
                nc.vector.reciprocal(out=recip, in_=rst_ps)
                for s in range(NSUB):
                    oo = epi.tile([P, C], f32, tag="oout", bufs=2)
                    nc.vector.tensor_scalar_mul(
                        out=oo, in0=o_ps[:, s, :], scalar1=recip[:, s : s + 1]
                    )
                    r0 = qb * QBLK + s * P
                    nc.gpsimd.dma_start(out=out[r0 : r0 + P, :], in_=oo)

            # ---- software-pipelined emission ----
            # Groups: q tiles first, then the 8 x-tile groups.  Loads for
            # group g+1 are emitted before group g's norm/transpose chain so
            # the in-order Sync queue never stalls the next group's loads,
            # and qb0's main iterations are interleaved group-wise so the
            # in-order ACT queue alternates prep work and exps.
            qb_psum[0] = (
                o_psum.tile([P, NSUB, C], f32, tag="o", name="o_ps0"),
                rs_psum.tile([1, QBLK], f32, tag="rs", name="rs_ps0"),
            )
            NXG = NK // NGRP
            # Variable-size prep groups (bigger groups amortize the ACT
            # Sqrt/Ln table loads), with main consumption one group behind
            # prep so transposes are in SBUF before the PE needs them.
            sizes = [8, 16, 16, 16, 8]
            starts = [0, 8, 24, 40, 56]
            NG = len(sizes)
            loads = {}
            loads[0] = emit_loads(x, starts[0], sizes[0])
            loads[1] = emit_loads(x, starts[1], sizes[1])
            prep_rest(loads.pop(0), starts[0], sizes[0], xnT, is_k_side=True)
            for gi in range(NG):
                pg = gi + 1
                if pg + 1 < NG:
                    loads[pg + 1] = emit_loads(
                        x, starts[pg + 1], sizes[pg + 1]
                    )
                if pg < NG:
                    prep_rest(
                        loads.pop(pg), starts[pg], sizes[pg], xnT,
                        is_k_side=True,
                    )
                for kb in range(starts[gi], starts[gi] + sizes[gi]):
                    main_iter(0, kb)
            epilogue(0)

            qb_psum[1] = (
                o_psum.tile([P, NSUB, C], f32, tag="o", name="o_ps1"),
                rs_psum.tile([1, QBLK], f32, tag="rs", name="rs_ps1"),
            )
            for kb in range(NK):
                main_iter(1, kb)
            epilogue(1)

    nc.compile()
    return nc


def kernel(**inputs):
    global _cached_nc
    from concourse import bass_utils

    x = np.ascontiguousarray(np.asarray(inputs["x"], dtype=np.float32))
    if _cached_nc is None:
        _cached_nc = _build()
    in_maps = [
        {"x": x if i == 0 else np.concatenate([x[i * QB :], x[: i * QB]])}
        for i in range(M)
    ]
    res = bass_utils.run_bass_kernel_spmd(_cached_nc, in_maps, core_ids=list(range(M)))
    return np.concatenate([res.results[i]["out"] for i in range(M)], axis=0)
